# revision 1
# baseline (speedup 1.0000x reference)
"""Trainium2 Bass kernel for a directed MPNN layer (8 NeuronCores, SPMD).

Reference computation (per edge e = (src, tgt)):
    msg  = relu(edge_hidden @ W_msg.T + b_msg)                     (E, H)
    agg  = segment_sum(msg, tgt, N)                                (N, H)
    excl[e] = sum msg[f] over f with (tgt_f, src_f) == (src_e, tgt_e)
    out[e]  = relu(x[src_e] @ Wx.T + edge_attr[e] @ Wa.T
                   + (agg[src_e] - excl[e]) @ Wm.T + b_upd)
  with W_upd = [Wx | Wa | Wm] split along columns (64 | 16 | 64).

Decomposition (no cross-core communication at all):
    node_term[v] = x[v] @ Wx.T + agg[v] @ Wm.T + b_upd
    out[e] = relu(node_term[src_e] + edge_attr[e] @ Wa.T - excl[e] @ Wm.T)

  Each core owns 5000 nodes. Edges are reverse pairs (e <-> e +/- E/2),
  so for out-edge e = rev(f), excl[e] = msg[f] (plus rare duplicate-pair
  corrections) and src_e = tgt_f. Sorting in-edges by tgt gives one
  stream that serves both passes:
    pass 1: msg(f) -> one-hot matmul segment-sum -> agg -> node_term
    pass 2: out[rev(f)] = relu(nt[tgt_f] + attrW[rev(f)] - msg(f)@Wm.T)
  node_term rows are delivered by a host-built one-hot matmul (U2), so
  there are no gathers. ~500 duplicate-pair corrections go through 128
  "special" node_term rows computed on device and a fix-up group whose
  outputs the host splices in.

Matmul dtypes: bf16 for the big per-edge streams (inputs host-cast),
float32r (1.6e-4) for node_term math. All accumulation is fp32 PSUM.
"""

import numpy as np
import ml_dtypes

import concourse.bacc as bacc
import concourse.bass as bass
import concourse.mybir as mybir
import concourse.tile as tile
from concourse.bass_utils import run_bass_kernel_spmd

F32 = mybir.dt.float32
F32R = mybir.dt.float32r
BF16 = mybir.dt.bfloat16
I32 = mybir.dt.int32
ALU = mybir.AluOpType
ACTF = mybir.ActivationFunctionType
NPBF = ml_dtypes.bfloat16

N = 40000
E = 800000
E2 = E // 2
H = 64
A = 16
NC = 8
P = 128

NPC = N // NC           # 5000 nodes per core
NBLK = 40               # 128-node blocks per core
NPC_PAD = NBLK * P      # 5120
SPEC_CAP = P            # special (correction) rows per core
NT_ROWS = NPC_PAD + SPEC_CAP

_CACHE = {}
_DEBUG_NT = False


def _build(k_blk: int):
    nch = NBLK * k_blk              # chunks per core (both passes)
    l1 = nch * P                    # padded edges per core
    assert nch % 2 == 0
    hch = nch // 2                  # chunks per partition-half of eh

    nc = bacc.Bacc("TRN2", target_bir_lowering=False, debug=False,
                   num_devices=NC)

    def inp(name, shape, dtype):
        return nc.dram_tensor(name, shape, dtype, kind="ExternalInput").ap()

    # eh (in-edges, tgt-sorted, feature-major): chunks 0..hch-1 on
    # partitions 0:64, chunks hch.. on partitions 64:128.
    eh2 = inp("eh2", [P, hch * P], BF16)
    tgt_rel = inp("tgt_rel", [P, nch], F32)
    attr_T = inp("attr_T", [A, l1], BF16)      # edge_attr of rev(f), T
    U2 = inp("U2", [P, l1], BF16)              # one-hot src_rel columns
    xT_own = inp("xT_own", [H, NPC_PAD], F32R)
    ehF_T = inp("ehF_T", [H, P], BF16)         # correction source rows
    ehRF_T = inp("ehRF_T", [H, P], BF16)       # eh[rev(affected e)], T
    attrF_T = inp("attrF_T", [A, P], BF16)
    Sneg = inp("Sneg", [P, P], F32R)
    didx = inp("didx", [P, 1], I32)
    Wmsg2 = inp("Wmsg2", [P, H], BF16)         # W_msg.T doubled (2x64)
    Wua = inp("Wua", [A, H], BF16)
    negWum = inp("negWum", [H, H], BF16)
    Wstack = inp("Wstack", [H + A, H], BF16)   # [negWum ; Wua]
    Wum = inp("Wum", [H, H], F32R)
    Wux = inp("Wux", [H, H], F32R)
    bupd = inp("bupd", [1, H], F32R)
    ones1 = inp("ones1", [1, P], F32R)
    ident = inp("ident", [P, P], BF16)
    iota4 = inp("iota4", [P, 4 * P], BF16)

    outT = nc.dram_tensor("outT", [H, l1 + P], F32, kind="ExternalOutput").ap()
    nt_own = nc.dram_tensor("nt_own", [NT_ROWS, P], BF16).ap()
    nt_dump = (nc.dram_tensor("nt_dump", [NT_ROWS, P], BF16,
                              kind="ExternalOutput").ap()
               if _DEBUG_NT else None)

    with tile.TileContext(nc) as tc:
        with (
            tc.tile_pool(name="const", bufs=1) as cst,
            tc.tile_pool(name="sb", bufs=3) as sb,
            tc.tile_pool(name="stage", bufs=3) as stg,
            tc.tile_pool(name="ps_msg", bufs=2, space="PSUM") as ps_msg,
            tc.tile_pool(name="ps_agg", bufs=2, space="PSUM") as ps_agg,
            tc.tile_pool(name="ps_m", bufs=2, space="PSUM") as ps_m,
            tc.tile_pool(name="ps_o", bufs=2, space="PSUM") as ps_o,
        ):
            def load_const(name, ap_in, shape, dtype):
                t = cst.tile(shape, dtype, tag=name)
                nc.sync.dma_start(t[:], ap_in[:])
                return t

            eh_sb = load_const("c_eh2", eh2, [P, hch * P], BF16)
            tgt_rel_sb = load_const("c_tgtrel", tgt_rel, [P, nch], F32)
            xT_sb = load_const("c_xt", xT_own, [H, NPC_PAD], F32R)
            Wmsg2_sb = load_const("c_wmsg2", Wmsg2, [P, H], BF16)
            Wua_sb = load_const("c_wua", Wua, [A, H], BF16)
            negWum_sb = load_const("c_nwum", negWum, [H, H], BF16)
            Wstack_sb = load_const("c_wstack", Wstack, [H + A, H], BF16)
            Wum_sb = load_const("c_wum", Wum, [H, H], F32R)
            Wux_sb = load_const("c_wux", Wux, [H, H], F32R)
            bupd_sb = load_const("c_bupd", bupd, [1, H], F32R)
            ones1_sb = load_const("c_ones1", ones1, [1, P], F32R)
            ident_sb = load_const("c_ident", ident, [P, P], BF16)
            iota4_sb = load_const("c_iota4", iota4, [P, 4 * P], BF16)
            Sneg_sb = load_const("c_sneg", Sneg, [P, P], F32R)
            didx_sb = load_const("c_didx", didx, [P, 1], I32)
            ehF_sb = load_const("c_ehf", ehF_T, [H, P], BF16)
            ehRF_sb = load_const("c_ehrf", ehRF_T, [H, P], BF16)
            attrF_sb = load_const("c_attrf", attrF_T, [A, P], BF16)

            def ehsl(ch, w=P):
                half, col = (0, ch) if ch < hch else (64, ch - hch)
                return eh_sb[half:half + H, col * P:col * P + w]

            def wmsl(ch):
                half = 0 if ch < hch else 64
                return Wmsg2_sb[half:half + H, :]

            # b_upd broadcast to 128 partitions via K=1 matmul
            ps_b = ps_agg.tile([P, H], F32, tag="agg")
            nc.tensor.matmul(ps_b[:], lhsT=ones1_sb[:],
                             rhs=bupd_sb[:],
                             start=True, stop=True)
            b_bcast = cst.tile([P, H], F32, tag="c_bb")
            nc.vector.tensor_copy(b_bcast[:], ps_b[:])

            # ---- pass 1: msg -> agg -> node_term, per 128-node block ----
            for b in range(NBLK):
                agg_ps = ps_agg.tile([H, P], F32, tag="agg")
                i = 0
                while i < k_blk:
                    gw = min(4, k_blk - i)
                    msg4_ps = ps_msg.tile([P, 4 * H], F32, tag="msg")
                    for j in range(gw):
                        ch = b * k_blk + i + j
                        nc.tensor.matmul(msg4_ps[:, j * H:(j + 1) * H],
                                         lhsT=ehsl(ch), rhs=wmsl(ch),
                                         start=True, stop=True)
                    msg4_sb = sb.tile([P, 4 * H], BF16, tag="msg_sb")
                    nc.vector.tensor_scalar(out=msg4_sb[:, :gw * H],
                                            in0=msg4_ps[:, :gw * H],
                                            scalar1=0.0, scalar2=None,
                                            op0=ALU.max)
                    ch0 = b * k_blk + i
                    t4_sb = sb.tile([P, 4 * P], BF16, tag="t_sb")
                    trs = tgt_rel_sb[:, ch0:ch0 + gw]
                    tr_bc = bass.AP(trs.tensor, trs.offset,
                                    trs.ap[:1] + [[1, gw], [0, P]])
                    nc.vector.tensor_tensor(out=t4_sb[:, :gw * P],
                                            in0=iota4_sb[:, :gw * P],
                                            in1=tr_bc, op=ALU.is_equal)
                    for j in range(gw):
                        nc.tensor.matmul(agg_ps[:],
                                         lhsT=msg4_sb[:, j * H:(j + 1) * H],
                                         rhs=t4_sb[:, j * P:(j + 1) * P],
                                         start=(i + j == 0),
                                         stop=(i + j == k_blk - 1))
                    i += gw
                aggT_sb = sb.tile([H, P], F32R, tag="aggT_sb")
                nc.vector.tensor_copy(aggT_sb[:], agg_ps[:])
                nt_ps = ps_msg.tile([P, H], F32, tag="msg")
                nc.tensor.matmul(nt_ps[:], lhsT=aggT_sb[:],
                                 rhs=Wum_sb[:],
                                 start=True, stop=False)
                nc.tensor.matmul(nt_ps[:],
                                 lhsT=xT_sb[:, b * P:(b + 1) * P],
                                 rhs=Wux_sb[:],
                                 start=False, stop=True)
                nt_sb = sb.tile([P, P], BF16, tag="nt_sb")
                nc.gpsimd.memset(nt_sb[:, H:], 0.0)
                nc.vector.tensor_tensor(out=nt_sb[:, 0:H], in0=nt_ps[:],
                                        in1=b_bcast[:], op=ALU.add)
                nc.sync.dma_start(nt_own[b * P:(b + 1) * P, :], nt_sb[:])

            # ---- special (correction) rows ----
            mF_ps = ps_m.tile([H, P], F32, tag="m")
            nc.tensor.matmul(mF_ps[:], lhsT=Wmsg2_sb[0:H, :], rhs=ehF_sb[:],
                             start=True, stop=True)
            mFT_sb = sb.tile([H, P], F32R, tag="mFT_sb")
            nc.vector.tensor_scalar(out=mFT_sb[:], in0=mF_ps[:], scalar1=0.0,
                                    scalar2=None, op0=ALU.max)
            mV_ps = ps_msg.tile([P, H], F32, tag="msg")
            nc.tensor.matmul(mV_ps[:], lhsT=mFT_sb[:],
                             rhs=Wum_sb[:],
                             start=True, stop=True)
            mV_sb = sb.tile([P, H], F32R, tag="mV_sb")
            nc.vector.tensor_copy(mV_sb[:], mV_ps[:])
            ntgD_sb = sb.tile([P, P], BF16, tag="ntgD_sb")
            nc.gpsimd.indirect_dma_start(
                out=ntgD_sb[:], out_offset=None, in_=nt_own[:],
                in_offset=bass.IndirectOffsetOnAxis(ap=didx_sb[:, 0:1], axis=0),
            )
            ntgD_f = sb.tile([P, H], F32, tag="ntgD_f")
            nc.vector.tensor_copy(ntgD_f[:], ntgD_sb[:, 0:H])
            spec_ps = ps_agg.tile([P, H], F32, tag="agg")
            nc.tensor.matmul(spec_ps[:], lhsT=Sneg_sb[:],
                             rhs=mV_sb[:],
                             start=True, stop=True)
            spec_sb = sb.tile([P, P], BF16, tag="spec_sb")
            nc.gpsimd.memset(spec_sb[:, H:], 0.0)
            nc.vector.tensor_tensor(out=spec_sb[:, 0:H], in0=spec_ps[:],
                                    in1=ntgD_f[:], op=ALU.add)
            nc.sync.dma_start(nt_own[NPC_PAD:NPC_PAD + SPEC_CAP, :],
                              spec_sb[:])

            if nt_dump is not None:
                nc.sync.dma_start(nt_dump[:], nt_own[:])

            # ---- pass 2: out[rev(f)] per block, groups of <=4 chunks ----
            # stacked rhs: partitions 0:64 = relu(msg_rev)T, 64:80 = attrT
            for b in range(NBLK):
                ntb_sb = sb.tile([P, P], BF16, tag="ntb")
                nc.sync.dma_start(ntb_sb[:], nt_own[b * P:(b + 1) * P, :])
                i = 0
                while i < k_blk:
                    gw = min(4, k_blk - i)          # chunks in this group
                    w = gw * P
                    ch0 = b * k_blk + i
                    c0 = ch0 * P
                    m_ps = ps_m.tile([H, 4 * P], F32, tag="m")
                    nc.tensor.matmul(m_ps[:, 0:w], lhsT=wmsl(ch0),
                                     rhs=ehsl(ch0, w), start=True, stop=True)
                    sx_sb = stg.tile([H + A, 4 * P], BF16, tag="sx")
                    nc.scalar.activation(sx_sb[0:H, 0:w], m_ps[:, 0:w],
                                         ACTF.Relu)
                    nc.scalar.dma_start(sx_sb[H:H + A, 0:w],
                                        attr_T[:, c0:c0 + w])
                    u2_sb = stg.tile([P, 4 * P], BF16, tag="u2")
                    nc.sync.dma_start(u2_sb[:, 0:w], U2[:, c0:c0 + w])
                    o_ps = ps_o.tile([H, 4 * P], F32, tag="o")
                    nc.tensor.matmul(o_ps[:, 0:w], lhsT=Wstack_sb[:],
                                     rhs=sx_sb[:, 0:w],
                                     start=True, stop=False)
                    nc.tensor.matmul(o_ps[:, 0:w], lhsT=ntb_sb[:, 0:H],
                                     rhs=u2_sb[:, 0:w],
                                     start=False, stop=True)
                    outT_sb = sb.tile([H, 4 * P], F32, tag="outT")
                    nc.vector.tensor_scalar(out=outT_sb[:, 0:w],
                                            in0=o_ps[:, 0:w], scalar1=0.0,
                                            scalar2=None, op0=ALU.max)
                    nc.scalar.dma_start(outT[:, c0:c0 + w], outT_sb[:, 0:w])
                    i += gw

            # ---- fix-up group for the corrected edges ----
            ntf_sb = sb.tile([P, P], BF16, tag="ntb")
            nc.sync.dma_start(ntf_sb[:], nt_own[NPC_PAD:NPC_PAD + P, :])
            mf_ps = ps_m.tile([H, 4 * P], F32, tag="m")
            nc.tensor.matmul(mf_ps[:, 0:P], lhsT=Wmsg2_sb[0:H, :],
                             rhs=ehRF_sb[:], start=True, stop=True)
            mfT_sb = sb.tile([H, 4 * P], BF16, tag="mrevT")
            nc.scalar.activation(mfT_sb[:, 0:P], mf_ps[:, 0:P], ACTF.Relu)
            of_ps = ps_o.tile([H, 4 * P], F32, tag="o")
            nc.tensor.matmul(of_ps[:, 0:P], lhsT=Wua_sb[:], rhs=attrF_sb[:],
                             start=True, stop=False)
            nc.tensor.matmul(of_ps[:, 0:P], lhsT=negWum_sb[:],
                             rhs=mfT_sb[:, 0:P], start=False, stop=False)
            nc.tensor.matmul(of_ps[:, 0:P], lhsT=ntf_sb[:, 0:H],
                             rhs=ident_sb[:], start=False, stop=True)
            outF_sb = sb.tile([H, 4 * P], F32, tag="outT")
            nc.vector.tensor_scalar(out=outF_sb[:, 0:P], in0=of_ps[:, 0:P],
                                    scalar1=0.0, scalar2=None, op0=ALU.max)
            nc.sync.dma_start(outT[:, l1:l1 + P], outF_sb[:, 0:P])

    nc.compile()
    return nc


def _host_prep(x, edge_attr, edge_hidden, W_msg, b_msg, W_upd, b_upd,
               edge_index):
    src = np.asarray(edge_index[0], dtype=np.int64)
    tgt = np.asarray(edge_index[1], dtype=np.int64)
    eh = np.asarray(edge_hidden, dtype=np.float32)
    ea = np.asarray(edge_attr, dtype=np.float32)
    x = np.asarray(x, dtype=np.float32)
    W_msg = np.asarray(W_msg, dtype=np.float32)
    b_msg = np.asarray(b_msg, dtype=np.float32)
    W_upd = np.asarray(W_upd, dtype=np.float32)
    b_upd = np.asarray(b_upd, dtype=np.float32)
    assert not np.any(b_msg), "nonzero b_msg unsupported by this build"

    # ---- tgt-sort & per-(core, block) runs ----
    order = np.argsort(tgt, kind="stable")
    tgt_s = tgt[order]
    bnd = np.empty((NC, NBLK, 2), np.int64)
    for c in range(NC):
        for b in range(NBLK):
            lo_n = c * NPC + b * P
            hi_n = min(c * NPC + (b + 1) * P, (c + 1) * NPC)
            bnd[c, b] = (np.searchsorted(tgt_s, lo_n, "left"),
                         np.searchsorted(tgt_s, hi_n, "left"))
    runs = bnd[:, :, 1] - bnd[:, :, 0]
    k_blk = int(np.ceil(runs.max() / P))
    if k_blk % 2:
        k_blk += 1                      # nch even for the 2-half packing
    nch = NBLK * k_blk
    l1 = nch * P
    hch = nch // 2

    # ---- exclusion groups (reference's int logic) ----
    keys = tgt * N + src
    q = src * N + tgt
    order2 = np.argsort(keys, kind="stable")
    sk = keys[order2]
    lo2 = np.searchsorted(sk, q, "left")
    hi2 = np.searchsorted(sk, q, "right")
    eids = np.arange(E, dtype=np.int64)
    rev = np.where(eids < E2, eids + E2, eids - E2)
    simple = (hi2 - lo2 == 1) & (order2[lo2] == rev)
    affected = np.where(~simple)[0]

    Wmsg_io = np.ascontiguousarray(W_msg.T)         # [in, out]
    Wmsg2 = np.concatenate([Wmsg_io, Wmsg_io], axis=0).astype(NPBF)
    iota_t = np.tile(np.arange(P, dtype=np.float32), (P, 1))

    in_maps = []
    meta = []
    for c in range(NC):
        gl = np.zeros(l1, np.int64)      # in-edge f per padded position
        trel = np.full(l1, -1.0, np.float32)
        valid = np.zeros(l1, bool)
        for b in range(NBLK):
            lo, hi = bnd[c, b]
            n = hi - lo
            base = b * k_blk * P
            gl[base:base + n] = order[lo:hi]
            trel[base:base + n] = tgt_s[lo:hi] - (c * NPC + b * P)
            valid[base:base + n] = True

        ehp = eh[gl].astype(NPBF)                     # [l1, 64]
        eh2 = np.empty((P, hch * P), NPBF)
        eh2[0:H] = ehp[:hch * P].T
        eh2[H:P] = ehp[hch * P:].T

        tgt_rel = np.ascontiguousarray(
            trel.reshape(nch, P).T)

        # pass 2: out-edge e = rev(f); src_e = tgt_f
        el = rev[gl]
        attr_Tc = np.ascontiguousarray(ea[el].T).astype(NPBF)
        u2 = np.zeros((P, l1), np.float32)
        pos = np.arange(l1)
        tr = trel.astype(np.int64)
        u2[tr[valid], pos[valid]] = 1.0
        u2 = u2.astype(NPBF)

        xpad = np.zeros((NPC_PAD, H), np.float32)
        n_x = min(NPC_PAD, N - c * NPC)
        xpad[:n_x] = x[c * NPC:c * NPC + n_x]

        # corrections
        aff_c = affected[(src[affected] >= c * NPC)
                         & (src[affected] < (c + 1) * NPC)]
        f_list, s_cols = [], []
        for d, e in enumerate(aff_c):
            for f in order2[lo2[e]:hi2[e]]:
                if f != rev[e]:
                    f_list.append(f)
                    s_cols.append(d)
        assert len(aff_c) <= SPEC_CAP, len(aff_c)
        assert len(f_list) <= P, len(f_list)
        ehF = np.zeros((P, H), np.float32)
        if f_list:
            ehF[:len(f_list)] = eh[np.asarray(f_list)]
        ehRF = np.zeros((P, H), np.float32)
        attrF = np.zeros((P, A), np.float32)
        if len(aff_c):
            ehRF[:len(aff_c)] = eh[rev[aff_c]]
            attrF[:len(aff_c)] = ea[aff_c]
        Sneg = np.zeros((P, P), np.float32)
        for fi, d in enumerate(s_cols):
            Sneg[fi, d] = -1.0
        didx = np.zeros((P, 1), np.int32)
        didx[:len(aff_c), 0] = src[aff_c] - c * NPC

        in_maps.append({
            "eh2": eh2,
            "tgt_rel": tgt_rel,
            "attr_T": attr_Tc,
            "U2": u2,
            "xT_own": np.ascontiguousarray(xpad.T),
            "ehF_T": np.ascontiguousarray(ehF.T).astype(NPBF),
            "ehRF_T": np.ascontiguousarray(ehRF.T).astype(NPBF),
            "attrF_T": np.ascontiguousarray(attrF.T).astype(NPBF),
            "Sneg": Sneg,
            "didx": didx,
            "Wmsg2": Wmsg2,
            "Wua": np.ascontiguousarray(W_upd[:, H:H + A].T).astype(NPBF),
            "negWum": np.ascontiguousarray(-W_upd[:, H + A:].T).astype(NPBF),
            "Wstack": np.concatenate(
                [-W_upd[:, H + A:].T, W_upd[:, H:H + A].T],
                axis=0).astype(NPBF),
            "Wum": np.ascontiguousarray(W_upd[:, H + A:].T),
            "Wux": np.ascontiguousarray(W_upd[:, :H].T),
            "bupd": np.ascontiguousarray(b_upd[None, :]),
            "ones1": np.ones((1, P), np.float32),
            "ident": np.eye(P, dtype=np.float32).astype(NPBF),
            "iota4": np.tile(iota_t, (1, 4)).astype(NPBF),
        })
        meta.append({"el": el, "valid": valid, "aff_c": aff_c})
    return in_maps, meta, k_blk


def kernel(**inputs) -> np.ndarray:
    in_maps, meta, k_blk = _host_prep(**inputs)
    if k_blk not in _CACHE:
        _CACHE[k_blk] = _build(k_blk)
    nc = _CACHE[k_blk]
    res = run_bass_kernel_spmd(nc, in_maps, core_ids=list(range(NC)))
    l1 = NBLK * k_blk * P
    out = np.empty((E, H), np.float32)
    for c in range(NC):
        oT = res.results[c]["outT"]
        m = meta[c]
        out[m["el"][m["valid"]]] = oT[:, :l1].T[m["valid"]]
    for c in range(NC):
        oT = res.results[c]["outT"]
        aff_c = meta[c]["aff_c"]
        if len(aff_c):
            out[aff_c] = oT[:, l1:l1 + len(aff_c)].T
    return out



# revision 7
# speedup vs baseline: 1.0966x; 1.0966x over previous
"""Trainium2 Bass kernel for a directed MPNN layer (8 NeuronCores, SPMD).

Reference computation (per edge e = (src, tgt)):
    msg  = relu(edge_hidden @ W_msg.T + b_msg)                     (E, H)
    agg  = segment_sum(msg, tgt, N)                                (N, H)
    excl[e] = sum msg[f] over f with (tgt_f, src_f) == (src_e, tgt_e)
    out[e]  = relu(x[src_e] @ Wx.T + edge_attr[e] @ Wa.T
                   + (agg[src_e] - excl[e]) @ Wm.T + b_upd)
  with W_upd = [Wx | Wa | Wm] split along columns (64 | 16 | 64).

Decomposition (no cross-core communication at all):
    node_term[v] = x[v] @ Wx.T + agg[v] @ Wm.T + b_upd
    out[e] = relu(node_term[src_e] + edge_attr[e] @ Wa.T - excl[e] @ Wm.T)

  Each core owns 5000 nodes, edges are reverse pairs (excl[rev f] =
  msg[f] up to rare duplicate-pair corrections), in-edges are tgt-sorted
  into 40 blocks of 128 nodes x k_blk chunks of 128 edges.

  Fused per-block sweep (vs the 2-phase baseline):
    - one-hot t4[edge, node] generated per chunk on DVE via
      tensor_scalar(iota, scalar1=tgt_rel, is_equal)  (4x mode)
    - one-hot u2[node, edge] generated on device too: GPSIMD
      partition_broadcast of the block's trel row + DVE
      tensor_scalar(trel_bc, scalar1=iotaP, is_equal)  -- no 28.8MB
      host U2 DMA.
    - node_term stays in SBUF between pass1(b) and pass2(b); the DRAM
      copy only feeds the rare duplicate-pair fixup.
    - attr is DMA'd per block directly into the sx stack; output is
      written bf16 (host casts to fp32).
"""

import numpy as np
import ml_dtypes

import concourse.bacc as bacc
import concourse.bass as bass
import concourse.mybir as mybir
import concourse.tile as tile
from concourse.bass_utils import run_bass_kernel_spmd

F32 = mybir.dt.float32
F32R = mybir.dt.float32r
BF16 = mybir.dt.bfloat16
I32 = mybir.dt.int32
ALU = mybir.AluOpType
ACTF = mybir.ActivationFunctionType
NPBF = ml_dtypes.bfloat16

N = 40000
E = 800000
E2 = E // 2
H = 64
A = 16
NC = 8
P = 128

NPC = N // NC           # 5000 nodes per core
NBLK = 40               # 128-node blocks per core
NPC_PAD = NBLK * P      # 5120
SPEC_CAP = P            # special (correction) rows per core
NT_ROWS = NPC_PAD + SPEC_CAP

_CACHE = {}


def _build(k_blk: int):
    nch = NBLK * k_blk              # chunks per core
    l1 = nch * P                    # padded edges per core
    hch = nch // 2                  # chunks per partition-half of eh
    assert hch == 20 * k_blk        # blocks never straddle halves
    lblk = k_blk * P                # edge slots per block

    nc = bacc.Bacc("TRN2", target_bir_lowering=False, debug=False,
                   num_devices=NC)

    def inp(name, shape, dtype):
        return nc.dram_tensor(name, shape, dtype, kind="ExternalInput").ap()

    # eh (in-edges, tgt-sorted, feature-major): chunks 0..hch-1 on
    # partitions 0:64, chunks hch.. on partitions 64:128.
    eh2 = inp("eh2", [P, hch * P], BF16)
    tgt_rel = inp("tgt_rel", [P, nch], F32)
    trel_rows = inp("trel_rows", [NBLK, lblk], BF16)
    attr_T = inp("attr_T", [A, l1], BF16)      # edge_attr of rev(f), T
    xT_own = inp("xT_own", [H, NPC_PAD], F32R)
    ehF_T = inp("ehF_T", [H, P], BF16)         # correction source rows
    ehRF_T = inp("ehRF_T", [H, P], BF16)       # eh[rev(affected e)], T
    attrF_T = inp("attrF_T", [A, P], BF16)
    Sneg = inp("Sneg", [P, P], F32R)
    didx = inp("didx", [P, 1], I32)
    Wmsg2 = inp("Wmsg2", [P, H], BF16)         # W_msg.T doubled (2x64)
    Wua = inp("Wua", [A, H], BF16)
    negWum = inp("negWum", [H, H], BF16)
    Wstack = inp("Wstack", [H + A, H], BF16)   # [negWum ; Wua]
    Wum = inp("Wum", [H, H], F32R)
    Wux = inp("Wux", [H, H], F32R)
    bupd = inp("bupd", [1, H], F32R)
    ones1 = inp("ones1", [1, P], F32R)
    ident = inp("ident", [P, P], BF16)
    iota1 = inp("iota1", [P, P], BF16)         # cols 0..127, all rows
    iotaP = inp("iotaP", [P, 1], F32)          # partition index

    outT = nc.dram_tensor("outT", [H, l1 + P], BF16,
                          kind="ExternalOutput").ap()
    nt_own = nc.dram_tensor("nt_own", [NT_ROWS, H], BF16).ap()

    with tile.TileContext(nc) as tc:
        with (
            tc.tile_pool(name="const", bufs=1) as cst,
            tc.tile_pool(name="sb", bufs=3) as sb,
            tc.tile_pool(name="sx", bufs=2) as sxp,
            tc.tile_pool(name="osb", bufs=2) as osb,
            tc.tile_pool(name="ntp", bufs=2) as ntp,
            tc.tile_pool(name="tbc", bufs=2) as tbc,
            tc.tile_pool(name="ps_msg", bufs=2, space="PSUM") as ps_msg,
            tc.tile_pool(name="ps_agg", bufs=2, space="PSUM") as ps_agg,
            tc.tile_pool(name="ps_m", bufs=2, space="PSUM") as ps_m,
            tc.tile_pool(name="ps_o", bufs=2, space="PSUM") as ps_o,
        ):
            def load_const(name, ap_in, shape, dtype):
                t = cst.tile(shape, dtype, tag=name)
                nc.sync.dma_start(t[:], ap_in[:])
                return t

            tgt_rel_sb = load_const("c_tgtrel", tgt_rel, [P, nch], F32)
            xT_sb = load_const("c_xt", xT_own, [H, NPC_PAD], F32R)
            Wmsg2_sb = load_const("c_wmsg2", Wmsg2, [P, H], BF16)
            Wua_sb = load_const("c_wua", Wua, [A, H], BF16)
            negWum_sb = load_const("c_nwum", negWum, [H, H], BF16)
            Wstack_sb = load_const("c_wstack", Wstack, [H + A, H], BF16)
            Wum_sb = load_const("c_wum", Wum, [H, H], F32R)
            Wux_sb = load_const("c_wux", Wux, [H, H], F32R)
            bupd_sb = load_const("c_bupd", bupd, [1, H], F32R)
            ones1_sb = load_const("c_ones1", ones1, [1, P], F32R)
            ident_sb = load_const("c_ident", ident, [P, P], BF16)
            iota1_sb = load_const("c_iota1", iota1, [P, P], BF16)
            iotaP_sb = load_const("c_iotap", iotaP, [P, 1], F32)
            Sneg_sb = load_const("c_sneg", Sneg, [P, P], F32R)
            didx_sb = load_const("c_didx", didx, [P, 1], I32)
            ehF_sb = load_const("c_ehf", ehF_T, [H, P], BF16)
            ehRF_sb = load_const("c_ehrf", ehRF_T, [H, P], BF16)
            attrF_sb = load_const("c_attrf", attrF_T, [A, P], BF16)

            # eh preload in 8 slices so block 0 can start early
            eh_sb = cst.tile([P, hch * P], BF16, tag="c_eh2")
            nsl = 8
            assert hch % nsl == 0
            slw = (hch // nsl) * P
            for s in range(nsl):
                nc.sync.dma_start(eh_sb[:, s * slw:(s + 1) * slw],
                                  eh2[:, s * slw:(s + 1) * slw])

            def ehsl(ch, w=P):
                half, col = (0, ch) if ch < hch else (64, ch - hch)
                return eh_sb[half:half + H, col * P:col * P + w]

            def wmsl(ch):
                half = 0 if ch < hch else 64
                return Wmsg2_sb[half:half + H, :]

            # b_upd broadcast to 128 partitions via K=1 matmul
            ps_b = ps_msg.tile([P, H], F32, tag="msg")
            nc.tensor.matmul(ps_b[:], lhsT=ones1_sb[:], rhs=bupd_sb[:],
                             start=True, stop=True)
            b_bcast = cst.tile([P, H], F32, tag="c_bb")
            nc.vector.tensor_copy(b_bcast[:], ps_b[:])

            # ---- fused per-block sweep ----
            for b in range(NBLK):
                c0 = b * k_blk                # first chunk of block
                # trel row broadcast for this block's u2 one-hots
                # (partition_broadcast requires the source at partition 0)
                trow = tbc.tile([1, lblk], BF16, tag="trow")
                nc.sync.dma_start(trow[:], trel_rows[b:b + 1, :])
                trel_bc = tbc.tile([P, lblk], BF16, tag="tbc")
                nc.gpsimd.partition_broadcast(trel_bc[:], trow[:])

                # pass 1: msg -> relu -> t4 -> agg
                agg_ps = ps_agg.tile([H, P], F32, tag="agg")
                i = 0
                while i < k_blk:
                    gw = min(4, k_blk - i)
                    msg_ps = ps_msg.tile([P, 4 * H], F32, tag="msg")
                    for j in range(gw):
                        ch = c0 + i + j
                        nc.tensor.matmul(msg_ps[:, j * H:(j + 1) * H],
                                         lhsT=ehsl(ch), rhs=wmsl(ch),
                                         start=True, stop=True)
                    msg_sb = sb.tile([P, 4 * H], BF16, tag="msg_sb")
                    nc.vector.tensor_scalar(out=msg_sb[:, :gw * H],
                                            in0=msg_ps[:, :gw * H],
                                            scalar1=0.0, scalar2=None,
                                            op0=ALU.max)
                    t4_sb = sb.tile([P, 4 * P], BF16, tag="t4_sb")
                    for j in range(gw):
                        ch = c0 + i + j
                        nc.vector.tensor_scalar(
                            out=t4_sb[:, j * P:(j + 1) * P],
                            in0=iota1_sb[:],
                            scalar1=tgt_rel_sb[:, ch:ch + 1],
                            scalar2=None, op0=ALU.is_equal)
                    for j in range(gw):
                        nc.tensor.matmul(agg_ps[:],
                                         lhsT=msg_sb[:, j * H:(j + 1) * H],
                                         rhs=t4_sb[:, j * P:(j + 1) * P],
                                         start=(i + j == 0),
                                         stop=(i + j == k_blk - 1))
                    i += gw

                # node_term for this block
                aggT_sb = sb.tile([H, P], F32R, tag="aggT_sb")
                nc.vector.tensor_copy(aggT_sb[:], agg_ps[:])
                nt_ps = ps_msg.tile([P, H], F32, tag="msg")
                nc.tensor.matmul(nt_ps[:], lhsT=aggT_sb[:], rhs=Wum_sb[:],
                                 start=True, stop=False)
                nc.tensor.matmul(nt_ps[:],
                                 lhsT=xT_sb[:, b * P:(b + 1) * P],
                                 rhs=Wux_sb[:], start=False, stop=True)
                nt_sb = ntp.tile([P, H], BF16, tag="nt_sb")
                nc.vector.tensor_tensor(out=nt_sb[:], in0=nt_ps[:],
                                        in1=b_bcast[:], op=ALU.add)
                nc.scalar.dma_start(nt_own[b * P:(b + 1) * P, :], nt_sb[:])

                # pass 2: out[rev(f)] = relu(nt[tgt_f] + attrW - mW)
                sx = sxp.tile([H + A, lblk], BF16, tag="sx")
                nc.scalar.dma_start(sx[H:H + A, :],
                                    attr_T[:, c0 * P:c0 * P + lblk])
                outsb = osb.tile([H, lblk], BF16, tag="outsb")
                i = 0
                while i < k_blk:
                    gw = min(4, k_blk - i)
                    w = gw * P
                    ch0 = c0 + i
                    gc = i * P                 # col offset within block
                    m_ps = ps_m.tile([H, 4 * P], F32, tag="m")
                    nc.tensor.matmul(m_ps[:, 0:w], lhsT=wmsl(ch0),
                                     rhs=ehsl(ch0, w), start=True,
                                     stop=True)
                    nc.scalar.activation(sx[0:H, gc:gc + w], m_ps[:, 0:w],
                                         ACTF.Relu)
                    u2_sb = sb.tile([P, 4 * P], BF16, tag="u2_sb")
                    nc.vector.tensor_scalar(out=u2_sb[:, 0:w],
                                            in0=trel_bc[:, gc:gc + w],
                                            scalar1=iotaP_sb[:, 0:1],
                                            scalar2=None,
                                            op0=ALU.is_equal)
                    o_ps = ps_o.tile([H, 4 * P], F32, tag="o")
                    nc.tensor.matmul(o_ps[:, 0:w], lhsT=Wstack_sb[:],
                                     rhs=sx[:, gc:gc + w],
                                     start=True, stop=False)
                    nc.tensor.matmul(o_ps[:, 0:w], lhsT=nt_sb[:],
                                     rhs=u2_sb[:, 0:w],
                                     start=False, stop=True)
                    nc.scalar.activation(outsb[:, gc:gc + w], o_ps[:, 0:w],
                                         ACTF.Relu)
                    i += gw
                nc.sync.dma_start(outT[:, c0 * P:c0 * P + lblk], outsb[:])

            # ---- special (correction) rows ----
            mF_ps = ps_m.tile([H, 4 * P], F32, tag="m")
            nc.tensor.matmul(mF_ps[:, 0:P], lhsT=Wmsg2_sb[0:H, :],
                             rhs=ehF_sb[:], start=True, stop=True)
            mFT_sb = sb.tile([H, P], F32R, tag="mFT_sb")
            nc.vector.tensor_scalar(out=mFT_sb[:], in0=mF_ps[:, 0:P],
                                    scalar1=0.0, scalar2=None, op0=ALU.max)
            mV_ps = ps_msg.tile([P, H], F32, tag="msg")
            nc.tensor.matmul(mV_ps[:], lhsT=mFT_sb[:], rhs=Wum_sb[:],
                             start=True, stop=True)
            mV_sb = sb.tile([P, H], F32R, tag="mV_sb")
            nc.vector.tensor_copy(mV_sb[:], mV_ps[:])
            ntgD_sb = sb.tile([P, H], BF16, tag="ntgD_sb")
            nc.gpsimd.indirect_dma_start(
                out=ntgD_sb[:], out_offset=None, in_=nt_own[:],
                in_offset=bass.IndirectOffsetOnAxis(ap=didx_sb[:, 0:1],
                                                    axis=0),
            )
            ntgD_f = sb.tile([P, H], F32, tag="ntgD_f")
            nc.vector.tensor_copy(ntgD_f[:], ntgD_sb[:])
            spec_ps = ps_msg.tile([P, H], F32, tag="msg")
            nc.tensor.matmul(spec_ps[:], lhsT=Sneg_sb[:], rhs=mV_sb[:],
                             start=True, stop=True)
            spec_sb = sb.tile([P, H], BF16, tag="spec_sb")
            nc.vector.tensor_tensor(out=spec_sb[:], in0=spec_ps[:],
                                    in1=ntgD_f[:], op=ALU.add)
            nc.scalar.dma_start(nt_own[NPC_PAD:NPC_PAD + SPEC_CAP, :],
                                spec_sb[:])

            # ---- fix-up group for the corrected edges ----
            ntf_sb = sb.tile([P, H], BF16, tag="ntf_sb")
            nc.sync.dma_start(ntf_sb[:], nt_own[NPC_PAD:NPC_PAD + P, :])
            mf_ps = ps_m.tile([H, 4 * P], F32, tag="m")
            nc.tensor.matmul(mf_ps[:, 0:P], lhsT=Wmsg2_sb[0:H, :],
                             rhs=ehRF_sb[:], start=True, stop=True)
            mfT_sb = sb.tile([H, P], BF16, tag="mrevT")
            nc.scalar.activation(mfT_sb[:], mf_ps[:, 0:P], ACTF.Relu)
            of_ps = ps_o.tile([H, 4 * P], F32, tag="o")
            nc.tensor.matmul(of_ps[:, 0:P], lhsT=Wua_sb[:], rhs=attrF_sb[:],
                             start=True, stop=False)
            nc.tensor.matmul(of_ps[:, 0:P], lhsT=negWum_sb[:],
                             rhs=mfT_sb[:], start=False, stop=False)
            nc.tensor.matmul(of_ps[:, 0:P], lhsT=ntf_sb[:],
                             rhs=ident_sb[:], start=False, stop=True)
            outF_sb = sb.tile([H, P], BF16, tag="outF")
            nc.vector.tensor_scalar(out=outF_sb[:], in0=of_ps[:, 0:P],
                                    scalar1=0.0, scalar2=None, op0=ALU.max)
            nc.sync.dma_start(outT[:, l1:l1 + P], outF_sb[:])

    nc.compile()
    return nc


def _host_prep(x, edge_attr, edge_hidden, W_msg, b_msg, W_upd, b_upd,
               edge_index):
    src = np.asarray(edge_index[0], dtype=np.int64)
    tgt = np.asarray(edge_index[1], dtype=np.int64)
    eh = np.asarray(edge_hidden, dtype=np.float32)
    ea = np.asarray(edge_attr, dtype=np.float32)
    x = np.asarray(x, dtype=np.float32)
    W_msg = np.asarray(W_msg, dtype=np.float32)
    b_msg = np.asarray(b_msg, dtype=np.float32)
    W_upd = np.asarray(W_upd, dtype=np.float32)
    b_upd = np.asarray(b_upd, dtype=np.float32)
    assert not np.any(b_msg), "nonzero b_msg unsupported by this build"

    # ---- tgt-sort & per-(core, block) runs ----
    order = np.argsort(tgt, kind="stable")
    tgt_s = tgt[order]
    bnd = np.empty((NC, NBLK, 2), np.int64)
    for c in range(NC):
        for b in range(NBLK):
            lo_n = c * NPC + b * P
            hi_n = min(c * NPC + (b + 1) * P, (c + 1) * NPC)
            bnd[c, b] = (np.searchsorted(tgt_s, lo_n, "left"),
                         np.searchsorted(tgt_s, hi_n, "left"))
    runs = bnd[:, :, 1] - bnd[:, :, 0]
    k_blk = int(np.ceil(runs.max() / P))
    if k_blk % 2:
        k_blk += 1                      # hch % 8 == 0 for eh slicing
    nch = NBLK * k_blk
    l1 = nch * P
    hch = nch // 2
    lblk = k_blk * P

    # ---- exclusion groups (reference's int logic) ----
    keys = tgt * N + src
    q = src * N + tgt
    order2 = np.argsort(keys, kind="stable")
    sk = keys[order2]
    lo2 = np.searchsorted(sk, q, "left")
    hi2 = np.searchsorted(sk, q, "right")
    eids = np.arange(E, dtype=np.int64)
    rev = np.where(eids < E2, eids + E2, eids - E2)
    simple = (hi2 - lo2 == 1) & (order2[lo2] == rev)
    affected = np.where(~simple)[0]

    Wmsg_io = np.ascontiguousarray(W_msg.T)         # [in, out]
    Wmsg2 = np.concatenate([Wmsg_io, Wmsg_io], axis=0).astype(NPBF)
    iota1 = np.tile(np.arange(P, dtype=np.float32), (P, 1)).astype(NPBF)
    iotaP = np.arange(P, dtype=np.float32).reshape(P, 1)

    in_maps = []
    meta = []
    for c in range(NC):
        gl = np.zeros(l1, np.int64)      # in-edge f per padded position
        trel = np.full(l1, -1.0, np.float32)
        valid = np.zeros(l1, bool)
        for b in range(NBLK):
            lo, hi = bnd[c, b]
            n = hi - lo
            base = b * k_blk * P
            gl[base:base + n] = order[lo:hi]
            trel[base:base + n] = tgt_s[lo:hi] - (c * NPC + b * P)
            valid[base:base + n] = True

        ehp = eh[gl].astype(NPBF)                     # [l1, 64]
        eh2 = np.empty((P, hch * P), NPBF)
        eh2[0:H] = ehp[:hch * P].T
        eh2[H:P] = ehp[hch * P:].T

        tgt_rel = np.ascontiguousarray(trel.reshape(nch, P).T)
        trel_rows = np.ascontiguousarray(
            trel.reshape(NBLK, lblk)).astype(NPBF)

        # pass 2: out-edge e = rev(f); src_e = tgt_f
        el = rev[gl]
        attr_Tc = np.ascontiguousarray(ea[el].T).astype(NPBF)

        xpad = np.zeros((NPC_PAD, H), np.float32)
        n_x = min(NPC_PAD, N - c * NPC)
        xpad[:n_x] = x[c * NPC:c * NPC + n_x]

        # corrections
        aff_c = affected[(src[affected] >= c * NPC)
                         & (src[affected] < (c + 1) * NPC)]
        f_list, s_cols = [], []
        for d, e in enumerate(aff_c):
            for f in order2[lo2[e]:hi2[e]]:
                if f != rev[e]:
                    f_list.append(f)
                    s_cols.append(d)
        assert len(aff_c) <= SPEC_CAP, len(aff_c)
        assert len(f_list) <= P, len(f_list)
        ehF = np.zeros((P, H), np.float32)
        if f_list:
            ehF[:len(f_list)] = eh[np.asarray(f_list)]
        ehRF = np.zeros((P, H), np.float32)
        attrF = np.zeros((P, A), np.float32)
        if len(aff_c):
            ehRF[:len(aff_c)] = eh[rev[aff_c]]
            attrF[:len(aff_c)] = ea[aff_c]
        Sneg = np.zeros((P, P), np.float32)
        for fi, d in enumerate(s_cols):
            Sneg[fi, d] = -1.0
        didx = np.zeros((P, 1), np.int32)
        didx[:len(aff_c), 0] = src[aff_c] - c * NPC

        in_maps.append({
            "eh2": eh2,
            "tgt_rel": tgt_rel,
            "trel_rows": trel_rows,
            "attr_T": attr_Tc,
            "xT_own": np.ascontiguousarray(xpad.T),
            "ehF_T": np.ascontiguousarray(ehF.T).astype(NPBF),
            "ehRF_T": np.ascontiguousarray(ehRF.T).astype(NPBF),
            "attrF_T": np.ascontiguousarray(attrF.T).astype(NPBF),
            "Sneg": Sneg,
            "didx": didx,
            "Wmsg2": Wmsg2,
            "Wua": np.ascontiguousarray(W_upd[:, H:H + A].T).astype(NPBF),
            "negWum": np.ascontiguousarray(-W_upd[:, H + A:].T).astype(NPBF),
            "Wstack": np.concatenate(
                [-W_upd[:, H + A:].T, W_upd[:, H:H + A].T],
                axis=0).astype(NPBF),
            "Wum": np.ascontiguousarray(W_upd[:, H + A:].T),
            "Wux": np.ascontiguousarray(W_upd[:, :H].T),
            "bupd": np.ascontiguousarray(b_upd[None, :]),
            "ones1": np.ones((1, P), np.float32),
            "ident": np.eye(P, dtype=np.float32).astype(NPBF),
            "iota1": iota1,
            "iotaP": iotaP,
        })
        meta.append({"el": el, "valid": valid, "aff_c": aff_c})
    return in_maps, meta, k_blk


def kernel(**inputs) -> np.ndarray:
    in_maps, meta, k_blk = _host_prep(**inputs)
    if k_blk not in _CACHE:
        _CACHE[k_blk] = _build(k_blk)
    nc = _CACHE[k_blk]
    res = run_bass_kernel_spmd(nc, in_maps, core_ids=list(range(NC)))
    l1 = NBLK * k_blk * P
    out = np.empty((E, H), np.float32)
    for c in range(NC):
        oT = np.asarray(res.results[c]["outT"], dtype=np.float32)
        m = meta[c]
        out[m["el"][m["valid"]]] = oT[:, :l1].T[m["valid"]]
    for c in range(NC):
        oT = np.asarray(res.results[c]["outT"], dtype=np.float32)
        aff_c = meta[c]["aff_c"]
        if len(aff_c):
            out[aff_c] = oT[:, l1:l1 + len(aff_c)].T
    return out
